# revision 5
# baseline (speedup 1.0000x reference)
"""Trainium2 Bass kernel for shifted-window correlation (27 shifts) + SE gate.

Reference computation (shapes hardcoded; B=1, C=16, W=80, H=96, D=112):
  corr[w,h,d,k] = mean_c x1[c,w,h,d] * x2[c, w+sx, h+sy, d+sz]   (zero-padded)
  s = mean_{w,h,d} corr;  g = sigmoid(w1 @ relu(w0 @ s + b0) + b1)
  out = corr * g

Strategy (8 cores, W sharded 10/core):
  - SBUF partition dim = (c:16, h8:8) where h8 = h // (H/8).
  - Shifts: sy via 3 h-shifted DMA loads of x2; sx as free-dim w offset
    (w halo in the loaded window); sz via even/odd d-parity loads so all
    bf16 tensor_tensor operands stay 4-byte aligned (DVE 2x mode).
  - Products on DVE (bf16, 2x); channel reduction on the PE via a fixed
    block-diagonal selection matmul (1/16 entries) accumulating 4 shifts
    per 32-partition PSUM column group; ACT drains PSUM -> SBUF with
    accum_out capturing squeeze partial sums; ungated corr spilled to DRAM.
  - Squeeze: per-core partials -> AllReduce (216 floats) -> on-device gate
    MLP (two tiny matmuls + relu/sigmoid, replicated per partition row).
  - Phase 2 re-reads spilled corr (prefetch hides in collective latency),
    multiplies by per-partition gate, writes out. Host reassembles +
    transposes to [1, W, H, D, 27].
"""

import sys
import types

import numpy as np
import ml_dtypes


def _install_ntff_hook_shim():
    """agent image's antenv lacks axon_hooks; needed only for trace=True."""
    if "antenv.axon_hooks" in sys.modules:
        return
    try:
        import antenv
        from trn_agent_boot.trn_boot import _ntff_profile_via_ctypes

        hook = _ntff_profile_via_ctypes("/opt/axon/libaxon_pjrt.so")
        mod = types.ModuleType("antenv.axon_hooks")
        ref = {"h": hook}
        mod.get_axon_ntff_profile_hook = lambda: ref["h"]
        mod.set_axon_ntff_profile_hook = lambda h: ref.__setitem__("h", h)
        sys.modules["antenv.axon_hooks"] = mod
        antenv.axon_hooks = mod
    except Exception:
        pass


_install_ntff_hook_shim()

import concourse.bacc as bacc  # noqa: E402
import concourse.tile as tile  # noqa: E402
import concourse.mybir as mybir  # noqa: E402
from concourse.bass_utils import run_bass_kernel_spmd  # noqa: E402

BF16 = mybir.dt.bfloat16
FP32 = mybir.dt.float32
AF = mybir.ActivationFunctionType
ALU = mybir.AluOpType

N_CORES = 8
C = 16
H8 = 8          # partition sub-dim over h
K = 27
MID = 6


class Cfg:
    def __init__(self, W=80, H=96, D=112, hblk_t=3, w_sl=4):
        assert H % H8 == 0
        self.W, self.H, self.D = W, H, D
        self.Wc = W // N_CORES          # w columns per core
        self.HB = H // H8               # hblk extent (free dim)
        assert self.HB % hblk_t == 0
        self.hblk_t = hblk_t            # hblk rows per chunk
        self.n_chunks = self.HB // hblk_t
        self.w_sl = w_sl                # w per matmul slice
        self.n_ws = (self.Wc + w_sl - 1) // w_sl
        self.De = D + 2                 # odd-copy d extent


# shift order matches reference: k = dx*9 + dy*3 + dz, s* = d*-1
SHIFTS = [(dx - 1, dy - 1, dz - 1)
          for dx in range(3) for dy in range(3) for dz in range(3)]


def _row_of(k, h8):
    """PSUM/spill partition row for (k, h8). Tile A: k 0..15, tile B: 16..26."""
    kk = k if k < 16 else k - 16
    base = 0 if k < 16 else 128
    return base + 32 * (kk // 4) + 8 * (kk % 4) + h8


def build_nc(cfg: Cfg):
    nc = bacc.Bacc("TRN2", target_bir_lowering=False, debug=False,
                   num_devices=N_CORES)
    HB, Wc, D, De = cfg.HB, cfg.Wc, cfg.D, cfg.De

    x1_d = nc.dram_tensor("x1", [128, HB, Wc, D], BF16, kind="ExternalInput")
    x2_d = {}
    for sy in (-1, 0, 1):
        x2_d[(sy, 0)] = nc.dram_tensor(f"x2_s{sy+1}_e", [128, HB, Wc + 2, D],
                                       BF16, kind="ExternalInput")
        x2_d[(sy, 1)] = nc.dram_tensor(f"x2_s{sy+1}_o", [128, HB, Wc + 2, De],
                                       BF16, kind="ExternalInput")
    sel_d = nc.dram_tensor("selmats", [128, 128], BF16, kind="ExternalInput")
    w0a_d = nc.dram_tensor("w0a", [128, MID], FP32, kind="ExternalInput")
    w0b_d = nc.dram_tensor("w0b", [88, MID], FP32, kind="ExternalInput")
    w1a_d = nc.dram_tensor("w1ra", [MID, 128], FP32, kind="ExternalInput")
    w1b_d = nc.dram_tensor("w1rb", [MID, 88], FP32, kind="ExternalInput")
    b0_d = nc.dram_tensor("b0c", [MID, 1], FP32, kind="ExternalInput")
    b1a_d = nc.dram_tensor("b1ra", [128, 1], FP32, kind="ExternalInput")
    b1b_d = nc.dram_tensor("b1rb", [88, 1], FP32, kind="ExternalInput")
    out_d = nc.dram_tensor("out", [216, HB, Wc, D], FP32, kind="ExternalOutput")

    n_drain = cfg.n_chunks * cfg.hblk_t * cfg.n_ws

    with tile.TileContext(nc) as tc:
        with (
            tc.tile_pool(name="const", bufs=1) as cpool,
            tc.tile_pool(name="x2p", bufs=2) as x2pool,
            tc.tile_pool(name="pp", bufs=4) as ppool,
            tc.tile_pool(name="stage", bufs=4) as spool,
            tc.tile_pool(name="ps", bufs=1, space="PSUM") as ps,
            tc.tile_pool(name="ph2", bufs=3) as p2pool,
            tc.tile_pool(name="dram", bufs=1, space="DRAM") as dram,
        ):
            # resident constants / inputs
            x1t = cpool.tile([128, HB, Wc, D], BF16)
            selt = cpool.tile([128, 128], BF16)
            w0at = cpool.tile([128, MID], FP32)
            w0bt = cpool.tile([88, MID], FP32)
            w1at = cpool.tile([MID, 128], FP32)
            w1bt = cpool.tile([MID, 88], FP32)
            b0t = cpool.tile([MID, 1], FP32)
            b1at = cpool.tile([128, 1], FP32)
            b1bt = cpool.tile([88, 1], FP32)
            accA = cpool.tile([128, n_drain], FP32)
            accB = cpool.tile([88, n_drain], FP32)

            nc.sync.dma_start(x1t[:], x1_d[:])
            nc.sync.dma_start(selt[:], sel_d[:])
            nc.sync.dma_start(w0at[:], w0a_d[:])
            nc.sync.dma_start(w0bt[:], w0b_d[:])
            nc.sync.dma_start(w1at[:], w1a_d[:])
            nc.sync.dma_start(w1bt[:], w1b_d[:])
            nc.sync.dma_start(b0t[:], b0_d[:])
            nc.sync.dma_start(b1at[:], b1a_d[:])
            nc.sync.dma_start(b1bt[:], b1b_d[:])

            spill = dram.tile([216, HB, Wc, D], FP32)

            drain_i = 0
            for ch in range(cfg.n_chunks):
                hb0 = ch * cfg.hblk_t
                # chunk's 6 shifted x2 windows
                x2t = {}
                for sy in (-1, 0, 1):
                    for par in (0, 1):
                        dd = D if par == 0 else De
                        t = x2pool.tile([128, cfg.hblk_t, Wc + 2, dd], BF16,
                                        tag=f"x2_{sy}_{par}")
                        nc.sync.dma_start(
                            t[:], x2_d[(sy, par)][:, hb0:hb0 + cfg.hblk_t, :, :])
                        x2t[(sy, par)] = t

                for j in range(cfg.hblk_t):
                    psA = [ps.tile([128, min(cfg.w_sl, Wc - ws * cfg.w_sl) * D],
                                   FP32, tag=f"psA{ws}", name=f"psA{ws}",
                                   padded_shape=[128, 512])
                           for ws in range(cfg.n_ws)]
                    psB = [ps.tile([128, min(cfg.w_sl, Wc - ws * cfg.w_sl) * D],
                                   FP32, tag=f"psB{ws}", name=f"psB{ws}",
                                   padded_shape=[128, 512])
                           for ws in range(cfg.n_ws)]
                    for k, (sx, sy, sz) in enumerate(SHIFTS):
                        par = 0 if sz == 0 else 1
                        doff = 0 if sz == 0 else sz + 1
                        src = x2t[(sy, par)]
                        p = ppool.tile([128, Wc, D], BF16, tag="P")
                        nc.vector.tensor_tensor(
                            p[:],
                            x1t[:, hb0 + j, :, :],
                            src[:, j, 1 + sx:1 + sx + Wc, doff:doff + D],
                            ALU.mult,
                        )
                        kk = k if k < 16 else k - 16
                        g32, v = kk // 4, kk % 4
                        nv = 4 if (k < 16 or g32 < 2) else 3
                        for ws in range(cfg.n_ws):
                            w0 = ws * cfg.w_sl
                            nw = min(cfg.w_sl, Wc - w0)
                            pst = psA[ws] if k < 16 else psB[ws]
                            nc.tensor.matmul(
                                pst[32 * g32:32 * g32 + 32, :],
                                selt[:, 32 * v:32 * v + 32],
                                p[:, w0:w0 + nw, :],
                                start=(v == 0), stop=(v == nv - 1),
                                tile_position=(0, 32 * g32),
                            )
                    for ws in range(cfg.n_ws):
                        w0 = ws * cfg.w_sl
                        nw = min(cfg.w_sl, Wc - w0)
                        nfree = nw * D
                        stA = spool.tile([128, nfree], FP32, tag="stA")
                        stB = spool.tile([88, nfree], FP32, tag="stB")
                        nc.scalar.activation(
                            stA[:], psA[ws][:, 0:nfree], AF.Copy,
                            accum_out=accA[:, drain_i:drain_i + 1])
                        nc.scalar.activation(
                            stB[:], psB[ws][0:88, 0:nfree], AF.Copy,
                            accum_out=accB[:, drain_i:drain_i + 1])
                        nc.sync.dma_start(
                            spill[0:128, hb0 + j, w0:w0 + nw, :], stA[:])
                        nc.sync.dma_start(
                            spill[128:216, hb0 + j, w0:w0 + nw, :], stB[:])
                        drain_i += 1

            # ---- squeeze partials + allreduce + gate ----
            pA = cpool.tile([128, 1], FP32)
            pB = cpool.tile([88, 1], FP32)
            nc.vector.tensor_reduce(pA[:], accA[:], mybir.AxisListType.X, ALU.add)
            nc.vector.tensor_reduce(pB[:], accB[:], mybir.AxisListType.X, ALU.add)
            cc_in = dram.tile([216, 1], FP32)
            cc_out = dram.tile([216, 1], FP32)
            nc.sync.dma_start(cc_in[0:128, :], pA[:])
            nc.sync.dma_start(cc_in[128:216, :], pB[:])
            nc.gpsimd.collective_compute(
                "AllReduce", ALU.add,
                replica_groups=[list(range(N_CORES))],
                ins=[cc_in[:].opt()],
                outs=[cc_out[:].opt()],
            )
            pAg = cpool.tile([128, 1], FP32)
            pBg = cpool.tile([88, 1], FP32)
            nc.sync.dma_start(pAg[:], cc_out[0:128, :])
            nc.sync.dma_start(pBg[:], cc_out[128:216, :])

            hps = ps.tile([MID, 1], FP32, tag="psA0", padded_shape=[128, 512])
            nc.tensor.matmul(hps[:], w0at[:], pAg[:], start=True, stop=False)
            nc.tensor.matmul(hps[:], w0bt[:], pBg[:], start=False, stop=True)
            hvec = cpool.tile([MID, 1], FP32)
            nc.scalar.activation(hvec[:], hps[:], AF.Relu, bias=b0t[:], scale=1.0)
            gpsA = ps.tile([128, 1], FP32, tag="psA1", padded_shape=[128, 512])
            gpsB = ps.tile([88, 1], FP32, tag="psA2", padded_shape=[128, 512])
            nc.tensor.matmul(gpsA[:], w1at[:], hvec[:], start=True, stop=True)
            nc.tensor.matmul(gpsB[:], w1bt[:], hvec[:], start=True, stop=True)
            gA = cpool.tile([128, 1], FP32)
            gB = cpool.tile([88, 1], FP32)
            nc.scalar.activation(gA[:], gpsA[:], AF.Sigmoid, bias=b1at[:], scale=1.0)
            nc.scalar.activation(gB[:], gpsB[:], AF.Sigmoid, bias=b1bt[:], scale=1.0)

            # ---- phase 2: gated writeout (per hblk row) ----
            for hb in range(HB):
                stA2 = p2pool.tile([128, Wc, D], FP32, tag="p2a")
                nc.sync.dma_start(stA2[:], spill[0:128, hb, :, :])
                nc.vector.tensor_scalar(stA2[:], stA2[:], gA[:], None, ALU.mult)
                nc.sync.dma_start(out_d[0:128, hb, :, :], stA2[:])
                stB2 = p2pool.tile([88, Wc, D], FP32, tag="p2b")
                nc.sync.dma_start(stB2[:], spill[128:216, hb, :, :])
                nc.vector.tensor_scalar(stB2[:], stB2[:], gB[:], None, ALU.mult)
                nc.sync.dma_start(out_d[128:216, hb, :, :], stB2[:])

    nc.compile()
    return nc


# ---------------- host-side prep / assembly ----------------

def make_gate_consts(w0, b0, w1, b1, cfg: Cfg):
    norm = 1.0 / (cfg.W * cfg.H * cfg.D)
    sel = np.zeros((128, 128), dtype=np.float32)
    for v in range(4):
        for c in range(C):
            for h8 in range(H8):
                sel[c * H8 + h8, 32 * v + 8 * v + h8] = 1.0 / 16
    w0 = np.asarray(w0, dtype=np.float32)
    w1 = np.asarray(w1, dtype=np.float32)
    b1 = np.asarray(b1, dtype=np.float32)
    w0a = np.zeros((128, MID), dtype=np.float32)
    w0b = np.zeros((88, MID), dtype=np.float32)
    w1ra = np.zeros((MID, 128), dtype=np.float32)
    w1rb = np.zeros((MID, 88), dtype=np.float32)
    b1ra = np.zeros((128, 1), dtype=np.float32)
    b1rb = np.zeros((88, 1), dtype=np.float32)
    for k in range(K):
        for h8 in range(H8):
            r = _row_of(k, h8)
            if k < 16:
                w0a[r, :] = w0[:, k] * norm
                w1ra[:, r] = w1[k, :]
                b1ra[r, 0] = b1[k]
            else:
                w0b[r - 128, :] = w0[:, k] * norm
                w1rb[:, r - 128] = w1[k, :]
                b1rb[r - 128, 0] = b1[k]
    return {
        "selmats": sel.astype(ml_dtypes.bfloat16),
        "w0a": w0a, "w0b": w0b, "w1ra": w1ra, "w1rb": w1rb,
        "b0c": np.asarray(b0, dtype=np.float32).reshape(MID, 1),
        "b1ra": b1ra, "b1rb": b1rb,
    }


def _fold(a, HB):
    # [C, w, H, D'] -> [(c h8), hblk, w, d]
    Cc, ww, hh, dd = a.shape
    a = a.reshape(Cc, ww, H8, HB, dd)
    a = np.ascontiguousarray(a.transpose(0, 2, 3, 1, 4))
    return a.reshape(C * H8, HB, ww, dd)


def make_inputs_per_core(x_1, x_2, w0, b0, w1, b1, cfg: Cfg):
    """x_1/x_2: [1, C, W, H, D] float32 -> list of per-core input dicts."""
    W, H, D, De = cfg.W, cfg.H, cfg.D, cfg.De
    Wc, HB = cfg.Wc, cfg.HB
    x1 = np.asarray(x_1)[0].transpose(0, 1, 2, 3)  # [C, W, H, D]
    x1 = x1.astype(ml_dtypes.bfloat16)
    x2 = np.asarray(x_2)[0].astype(ml_dtypes.bfloat16)
    # padded x2: w +-1, h +-1, d in [-1, D+1)
    x2p = np.zeros((C, W + 2, H + 2, D + 2), dtype=ml_dtypes.bfloat16)
    x2p[:, 1:W + 1, 1:H + 1, 1:D + 1] = x2

    consts = make_gate_consts(w0, b0, w1, b1, cfg)
    in_maps = []
    for ci in range(N_CORES):
        ws = ci * Wc
        m = dict(consts)
        m["x1"] = _fold(x1[:, ws:ws + Wc, :, :], HB)
        for sy in (-1, 0, 1):
            hsl = slice(1 + sy, 1 + sy + H)
            wsl = slice(ws, ws + Wc + 2)
            m[f"x2_s{sy+1}_e"] = _fold(x2p[:, wsl, hsl, 1:1 + D], HB)
            m[f"x2_s{sy+1}_o"] = _fold(x2p[:, wsl, hsl, 0:De], HB)
        in_maps.append(m)
    return in_maps


def assemble_output(results, cfg: Cfg):
    W, H, D = cfg.W, cfg.H, cfg.D
    Wc, HB = cfg.Wc, cfg.HB
    rows = np.empty((K, H8), dtype=np.int64)
    for k in range(K):
        for h8 in range(H8):
            rows[k, h8] = _row_of(k, h8)
    out = np.empty((W, H, D, K), dtype=np.float32)
    for ci, r in enumerate(results):
        o = np.asarray(r["out"]).reshape(216, HB, Wc, D)
        core = o[rows]                        # [K, H8, HB, Wc, D]
        core = core.transpose(3, 1, 2, 4, 0)  # [Wc, H8, HB, D, K]
        out[ci * Wc:(ci + 1) * Wc] = core.reshape(Wc, H, D, K)
    return out[None]


_CACHE = {}
TRACE = False           # test harness can set kernel.TRACE = True


def kernel(x_1, x_2, w0, b0, w1, b1):
    cfg = Cfg()
    if "nc" not in _CACHE:
        _CACHE["nc"] = build_nc(cfg)
    nc = _CACHE["nc"]
    in_maps = make_inputs_per_core(x_1, x_2, w0, b0, w1, b1, cfg)
    res = run_bass_kernel_spmd(nc, in_maps, core_ids=list(range(N_CORES)),
                               trace=TRACE)
    _CACHE["last_res"] = res
    return assemble_output(res.results, cfg)
